# revision 38
# baseline (speedup 1.0000x reference)
"""MoE (top-2 of 8 experts) Trainium2 kernel, expert-parallel across 8 NeuronCores.

Strategy (matches the expert-parallel sharding hint):
  - Host computes the router (logits -> top-2 -> softmax) and performs the
    token all-to-all: tokens are gathered per expert, padded to a common
    capacity C, and each core gets one expert's tokens + that expert's
    W1/b1/W2 weights.
  - Each core runs a Bass/Tile kernel computing
        y = gelu_exact(x @ W1 + b1) @ W2
    in bf16 (fp32 PSUM accumulate, ~3e-3 rel err, well under the 2e-2 gate).
  - Host scatter-adds the per-expert outputs back with the routing weights
    and adds sum_k w_k * b2[e_k] (folding b2 into the host combine).

Per-core dataflow (two phases, PE never idles between them):
  Phase A (h = gelu(x @ W1 + b1)): stationary = W1 128x128 blocks streamed
  from HBM, moving = x token blocks, all N=512 wide so the 150ns LDWEIGHTS
  stays hidden under 216ns matmuls; PSUM [f, 512 tok]; exact GELU +
  per-partition bias b1 fused into one ScalarE activation per tile; h kept
  RESIDENT in SBUF as bf16. Startup is bandwidth-walled (~240GB/s/core
  while all 8 cores pull their first bytes), so the critical transfers are
  demand-ordered across the sync/gpsimd/scalar DMA queues: x block 0 rides
  both rings as two dk-half TILES (the first 4 matmuls start when the lo
  half lands), w1_0 on the scalar ring, then a W1 ladder alternating
  rings; the first K=10 f-tiles run block 0 only, then their deferred
  block-1 groups (W1 held resident, W2 prefetch pulled into this no-new-
  data window), so x block 1's deadline sits past the bandwidth wall.
  Phase B (y = h @ W2): W2 fully resident in one SBUF tile (prefetched
  behind the W1 stream); stationary = h blocks [128 f, 128 tok], moving =
  W2 rows [128 f, 512 d]; each token pair's y accumulates over all 32
  f-tiles in dedicated PSUM banks, then drains (ScalarE+VectorE halves in
  parallel -> bf16 -> DMA) while the next pair accumulates; the last tile
  runs as three pieces (512/256/256 d-cols) in separate recycled PSUM
  tiles so only a 64KB drain+DMA sits in the program tail.
"""

import numpy as np
import ml_dtypes

import concourse.mybir as mybir
import concourse.tile as tile
from concourse import bacc
from concourse.bass_utils import run_bass_kernel_spmd

P = 128
D = 1024
F = 4096
E = 8
TOP_K = 2
DK = D // P   # 8 contraction tiles for GEMM1
FT = F // P   # 32 f tiles
N_CORES = 8

BF16 = ml_dtypes.bfloat16

_F32 = mybir.dt.float32
_BF16 = mybir.dt.bfloat16

_compiled = {}  # C -> Bacc program


def _token_chunks(C):
    """Split C into 512-token chunks (PSUM-bank-width moving dim)."""
    chunks = []
    off = 0
    while off < C:
        cn = min(512, C - off)
        chunks.append((off, cn))
        off += cn
    return chunks


def _build(C):
    assert C % 256 == 0
    TT = C // P   # token tiles for GEMM2
    blocks = _token_chunks(C)   # 512-token blocks: h layout / phase B
    K = min(10, FT)             # f-tiles that run before x block 1 arrives
    nc = bacc.Bacc(None, target_bir_lowering=False)

    # x layout: dk-major 512-token blocks ([P, DK, 512] each, contiguous
    # per block so startup DMAs can slice dk-halves of block 0).
    xt_d = nc.dram_tensor("xt", [P, DK * C], _BF16, kind="ExternalInput")
    w1_d = nc.dram_tensor("w1", [FT, P, DK, P], _BF16, kind="ExternalInput")
    w2_d = nc.dram_tensor("w2", [FT, P, D], _BF16, kind="ExternalInput")
    b1_d = nc.dram_tensor("b1", [P, FT], _F32, kind="ExternalInput")
    y_d = nc.dram_tensor("y", [TT // 2, P, 2, D], _BF16, kind="ExternalOutput")

    with tile.TileContext(nc) as tc:
        with (
            tc.tile_pool(name="xpool", bufs=1) as xpool,
            tc.tile_pool(name="cpool", bufs=1) as cpool,
            tc.tile_pool(name="w1pool", bufs=1) as w1pool,
            tc.tile_pool(name="w2pool", bufs=1) as w2pool,
            tc.tile_pool(name="hpool", bufs=1) as hpool,
            tc.tile_pool(name="ypool", bufs=2) as ypool,
            # 4 GEMM1 banks absorb ScalarE GELU / supply jitter three groups
            # deep; the sequential phase B reuses an acc tile only every
            # ~27us, so 2 ypsum tiles (4 banks) suffice. 4 + 4 = all 8 banks.
            tc.tile_pool(name="hpsum", bufs=4, space="PSUM") as hpsum,
            tc.tile_pool(name="ypsum", bufs=2, space="PSUM") as ypsum,
        ):
            # All inputs ride the sync DMA ring in demand order (the ring is
            # FIFO; emission order is preserved for these uniform triggers).
            # gate() pins a trigger behind the first x chunk via a WAW write
            # into its destination, so the list-scheduler cannot hoist it.
            def gate(dst_corner, src_tile):
                nc.vector.tensor_copy(dst_corner, src_tile[:, 0, 0:2])

            w1_live = K + 2  # first K held through their deferred groups
            solo = {}

            def w1_dma(ft, gated=False, eng=None, gate_src=None, split=False):
                t = w1pool.tile(
                    [P, DK, P], _BF16, tag="w1t", bufs=w1_live, name=f"w1s{ft}"
                )
                if gated or gate_src is not None:
                    gate(t[:, 0, 0:2], gate_src if gate_src is not None else xt_sb[0])
                if split:
                    # startup-critical: halves ride both DMA rings in parallel
                    h = DK // 2
                    nc.sync.dma_start(out=t[:, 0:h], in_=w1_d[ft][:, 0:h])
                    nc.gpsimd.dma_start(out=t[:, h:DK], in_=w1_d[ft][:, h:DK])
                else:
                    (eng or nc.sync).dma_start(out=t[:], in_=w1_d[ft])
                solo[ft] = t

            w2_sb = w2pool.tile([P, FT, D], _BF16, name="w2sb")
            w2_fill = [0]

            def w2_dma(gated=False):
                k = w2_fill[0]
                if k < FT:
                    if gated:
                        gate(w2_sb[:, k, 0:2], xt_sb[0])
                    nc.sync.dma_start(out=w2_sb[:, k], in_=w2_d[k])
                    w2_fill[0] = k + 1

            h_sb = [
                hpool.tile([P, FT, cn], _BF16, tag=f"hc{ci}", name=f"hc{ci}")
                for ci, (_, cn) in enumerate(blocks)
            ]

            # chunk list for GEMM1: (x-dram-offset, width, h-offset).
            # All chunks are full 512-token blocks: ring-split startup DMA
            # gets block 0 on-chip by ~11.5us, and N=512 keeps every group's
            # LDWEIGHTS hidden under 216ns matmuls (256-wide groups are
            # LDWEIGHTS-bound: 151ns load > 109ns matmul).
            xchunks = [(DK * c0, cn, c0) for c0, cn in blocks]
            xt_sb = []
            x0_hi = [None]  # dk 4..7 of block 0 (separate tile: see x_dma)

            def x_dma(si, gated=False, split=False, dk_tiles=False):
                o, cn, _ = xchunks[si]
                if dk_tiles:
                    # Block 0 as TWO dk-half tiles so the first group's
                    # dk 0..3 matmuls start when the lo half lands (~2.3us
                    # before the full block would; tile-granular dependency
                    # tracking would otherwise hold them for all of x0).
                    # Each half still ring-splits for parallel transfer.
                    hw = DK // 2 * cn
                    lo = xpool.tile([P, DK // 2, cn], _BF16, tag="xt0lo", name="xt0lo")
                    hi = xpool.tile([P, DK // 2, cn], _BF16, tag="xt0hi", name="xt0hi")
                    qw = hw // 2
                    nc.sync.dma_start(out=lo[:, 0 : DK // 4], in_=xt_d[:, o : o + qw])
                    nc.gpsimd.dma_start(
                        out=lo[:, DK // 4 :], in_=xt_d[:, o + qw : o + hw]
                    )
                    nc.sync.dma_start(
                        out=hi[:, 0 : DK // 4], in_=xt_d[:, o + hw : o + hw + qw]
                    )
                    nc.gpsimd.dma_start(
                        out=hi[:, DK // 4 :], in_=xt_d[:, o + hw + qw : o + 2 * hw]
                    )
                    xt_sb.append(lo)
                    x0_hi[0] = hi
                    return
                t = xpool.tile([P, DK, cn], _BF16, tag=f"xt{si}", name=f"xt{si}")
                if gated:
                    gate(t[:, 0, 0:2], xt_sb[0])
                if split:
                    # startup-critical: halves ride both DMA rings in parallel
                    hw = DK // 2 * cn
                    nc.sync.dma_start(out=t[:, 0 : DK // 2], in_=xt_d[:, o : o + hw])
                    nc.gpsimd.dma_start(
                        out=t[:, DK // 2 : DK], in_=xt_d[:, o + hw : o + 2 * hw]
                    )
                else:
                    nc.sync.dma_start(out=t[:], in_=xt_d[:, o : o + DK * cn])
                xt_sb.append(t)

            def x_mov(si, dk):
                """Moving operand for (chunk si, contraction tile dk)."""
                if si == 0 and x0_hi[0] is not None:
                    t = xt_sb[0] if dk < DK // 2 else x0_hi[0]
                    return t[:, dk % (DK // 2), :]
                return xt_sb[si][:, dk, :]

            # Startup emission. Each DMA trigger costs ~0.6-0.7us of QUEUE
            # time, so the critical first wave is spread across FOUR queues
            # (scalar + vector are idle at startup and can trigger DMAs too):
            #   sync:   x0 lo-half      gpsimd: x0 hi-half
            #   scalar: w1_0 (then the dummy-GELU table load)
            #   vector: b1, w1_1 (then the warm memset + gates)
            # The first group's inputs (x0+w1_0+b1 = 1.28MB) then complete
            # at the aggregate-bandwidth floor (~11.5us); later tiles queue
            # FIFO behind them on the two rings.
            # Warm-tile memset FIRST on the gpsimd queue (runs ~6.2us, before
            # its DMA trigger) so the PE warm-up starts during engine init.
            warm = cpool.tile([P, 512], _BF16, tag="warm")
            nc.gpsimd.memset(warm[:], 0.0)
            w1_dma(0, eng=nc.scalar)
            b1_sb = cpool.tile([P, FT], _F32)
            nc.scalar.dma_start(out=b1_sb[:], in_=b1_d[:])
            x_dma(0, dk_tiles=True)
            # W1 ladder BEHIND x0's halves: w1_1 is split across both rings
            # (it is due only ~1.7us after x0 lands, sooner than a whole
            # tile can follow x0 on one ring); the rest alternate rings,
            # landing every ~1.1us (two rings at ~120GB/s each under the
            # 8-core startup contention) vs the 1.73us/tile consumption.
            # K=10 pushes x block 1's deadline past the ~240GB/s wall.
            w1_dma(1, split=True)
            for ft in range(2, K):
                w1_dma(ft, eng=nc.sync if ft % 2 == 0 else nc.gpsimd)
            # Dummy GELU on scratch: pulls ScalarE's ~1.3us ACT_TABLE_LOAD
            # into the startup DMA wait (scalar queue: w1_0 trigger, then
            # this). Otherwise the FIRST real GELU pays it, holds an hpsum
            # buffer longer, and stalls the PE through the rotation.
            scratch = cpool.tile([P, 16], _BF16, tag="scr")
            nc.scalar.activation(
                scratch[:],
                warm[:, 0:16],
                mybir.ActivationFunctionType.Gelu,
                bias=warm[:, 0:1],
                scale=1.0,
            )
            # PE warm-up: dummy zero matmuls with no DMA deps run during the
            # initial input-DMA wait, so the HAM clock gate reaches 2.4 GHz
            # before the real stream starts; sized to end at the measured
            # block-0 arrival (~11.5us).
            # Sized to cover the SLOWEST core's data arrival (~13us): the
            # max-core sets the graded time, and an idle gap before its
            # first real matmul also resets the HAM clock ramp (~2us of
            # 379ns mid-pstate matmuls). Fast cores just queue briefly.
            WARM = (9, 8)
            for r, nw in enumerate(WARM):
                pw = hpsum.tile([P, 512], _F32, tag="ph", name=f"pw{r}")
                for k in range(nw):
                    nc.tensor.matmul(
                        pw[:], warm[:, :P], warm[:], start=(k == 0), stop=(k == nw - 1)
                    )
            # x block 1 follows the W1 ladder on both rings (ring FIFO
            # orders the transfers; the gate pins emission order against
            # list-scheduler hoisting).
            if len(xchunks) > 1:
                x_dma(1, split=True, gated=True)

            def gemm1_group(ft, si):
                _, cn, h0 = xchunks[si]
                ph = hpsum.tile([P, 512], _F32, tag="ph")
                for dk in range(DK):
                    nc.tensor.matmul(
                        ph[:, :cn],
                        solo[ft][:, dk, :],
                        x_mov(si, dk),
                        start=(dk == 0),
                        stop=(dk == DK - 1),
                    )
                nc.scalar.activation(
                    h_sb[h0 // 512][:, ft, h0 % 512 : h0 % 512 + cn],
                    ph[:, :cn],
                    mybir.ActivationFunctionType.Gelu,
                    bias=b1_sb[:, ft : ft + 1],
                    scale=1.0,
                )

            # Phase A order: the first K f-tiles run block 0 while block 1
            # streams in, then their deferred block-1 groups (W1 tiles held
            # resident); the rest run f-tile-major over both blocks so each
            # W1 tile is streamed exactly once.
            NB = len(xchunks)
            order = [(ft, 0) for ft in range(K)]
            for b in range(1, NB):
                order += [(ft, b) for ft in range(K)]
            order += [(ft, b) for ft in range(K, FT) for b in range(NB)]

            seen = set(ft for ft, _ in order[:K])
            for ft, si in order:
                if ft not in solo:
                    w1_dma(ft, gated=True)
                if ft not in seen:
                    seen.add(ft)
                    # W1 lookahead + W2 prefetch ride the same ring.
                    la = ft + 2
                    if la < FT and la not in solo:
                        w1_dma(la, gated=True)
                    w2_dma(gated=w2_fill[0] < 2)
                    w2_dma(gated=w2_fill[0] < 2)
                elif si >= 1 and ft < K:
                    # deferred-block groups consume no new data: use their
                    # window to pull the W2 prefetch forward
                    w2_dma(gated=w2_fill[0] < 2)
                gemm1_group(ft, si)
            while w2_fill[0] < FT:
                w2_dma()

            # Phase B: token tiles, full 32-step PSUM accumulation each.
            # The two tiles of a pair run SEQUENTIALLY (not ft-interleaved),
            # so each tile's drain + y DMA overlaps the next tile's 64-MM
            # accumulation; only the very last tile's drain+DMA lands in the
            # program tail.
            for tq in range(TT // 2):
                ci = (tq * 2 * P) // 512  # block holding this token pair
                cb = tq * 2 * P - blocks[ci][0]  # base token within block
                ysb = ypool.tile([P, 2, D], _BF16, tag="ysb")
                for tt2 in range(2):
                    acc = ypsum.tile([P, D], _F32, tag="py", name=f"py{tq}_{tt2}")
                    eng = nc.sync if (tq * 2 + tt2) % 2 == 0 else nc.gpsimd
                    if tq == TT // 2 - 1 and tt2 == 1:
                        # Final token tile: d-half 0, then two d-quarters as
                        # separate sequential accumulation groups, so the
                        # program-tail drain + y DMA is only 64KB (the drains
                        # of the earlier pieces hide under later matmuls).
                        # Each piece gets its OWN PSUM tile (recycled from the
                        # idle phase-A pool): slicing one shared acc tile puts
                        # a false WAR between piece N's drain-read and piece
                        # N+1's first matmul (~0.8us PE stall each, measured).
                        pieces = [(0, 512), (512, 256), (768, 256)]
                        for pi, (d0, dn) in enumerate(pieces):
                            pt = hpsum.tile([P, 512], _F32, tag="ph", name=f"pyf{pi}")
                            for ft in range(FT):
                                hblk = h_sb[ci][
                                    :, ft, cb + tt2 * P : cb + (tt2 + 1) * P
                                ]
                                nc.tensor.matmul(
                                    pt[:, :dn],
                                    hblk,
                                    w2_sb[:, ft, d0 : d0 + dn],
                                    start=(ft == 0),
                                    stop=(ft == FT - 1),
                                )
                            if pi % 2 == 0:
                                nc.scalar.activation(
                                    ysb[:, tt2, d0 : d0 + dn],
                                    pt[:, :dn],
                                    mybir.ActivationFunctionType.Copy,
                                )
                                nc.sync.dma_start(
                                    out=y_d[tq, :, tt2, d0 : d0 + dn],
                                    in_=ysb[:, tt2, d0 : d0 + dn],
                                )
                            else:
                                nc.vector.tensor_copy(
                                    ysb[:, tt2, d0 : d0 + dn],
                                    pt[:, :dn],
                                )
                                nc.gpsimd.dma_start(
                                    out=y_d[tq, :, tt2, d0 : d0 + dn],
                                    in_=ysb[:, tt2, d0 : d0 + dn],
                                )
                        continue
                    for ft in range(FT):
                        hblk = h_sb[ci][:, ft, cb + tt2 * P : cb + (tt2 + 1) * P]
                        for dh in range(2):
                            nc.tensor.matmul(
                                acc[:, dh * 512 : (dh + 1) * 512],
                                hblk,
                                w2_sb[:, ft, dh * 512 : (dh + 1) * 512],
                                start=(ft == 0),
                                stop=(ft == FT - 1),
                            )
                    # Drain the two PSUM banks in parallel on Scalar+Vector,
                    # then ship this token tile immediately.
                    nc.scalar.activation(
                        ysb[:, tt2, :512],
                        acc[:, :512],
                        mybir.ActivationFunctionType.Copy,
                    )
                    nc.vector.tensor_copy(ysb[:, tt2, 512:], acc[:, 512:])
                    eng.dma_start(out=y_d[tq, :, tt2, :], in_=ysb[:, tt2, :])

    nc.compile()
    return nc


def _route(xf, Wr, br):
    """Host router: exact top-2 + softmax weights (float64 for stable order)."""
    logits = xf.astype(np.float64) @ Wr.astype(np.float64) + br.astype(np.float64)
    order = np.argsort(-logits, axis=1, kind="stable")
    top2 = order[:, :TOP_K]  # [T, 2]
    v = np.take_along_axis(logits, top2, axis=1)
    v = v - v.max(axis=1, keepdims=True)
    ev = np.exp(v)
    rw = (ev / ev.sum(axis=1, keepdims=True)).astype(np.float32)  # [T, 2]
    return top2, rw


def _run(x, Wr, br, W1, b1, W2, b2, trace=False):
    B, S, d = x.shape
    T = B * S
    xf = np.ascontiguousarray(np.asarray(x, dtype=np.float32).reshape(T, d))

    top2, rw = _route(xf, Wr, br)

    token_lists = []
    weight_lists = []
    for e in range(E):
        in_slot0 = top2[:, 0] == e
        in_slot1 = top2[:, 1] == e
        toks = np.nonzero(in_slot0 | in_slot1)[0]
        w = np.where(in_slot0[toks], rw[toks, 0], rw[toks, 1]).astype(np.float32)
        token_lists.append(toks)
        weight_lists.append(w)

    # Capacity: balanced mean (rounded up to 256), capped by the SBUF
    # working set (x + h + W2 are resident). Pairs beyond it are computed
    # on the host - cheap for near-balanced routing.
    C = max(256, min(1024, -(-(T * TOP_K // E) // 256) * 256))
    spill_lists = [(t[C:], w[C:]) for t, w in zip(token_lists, weight_lists)]
    token_lists = [t[:C] for t in token_lists]
    weight_lists = [w[:C] for w in weight_lists]

    if C not in _compiled:
        _compiled[C] = _build(C)
    nc = _compiled[C]

    # Per-expert weight layouts (see _build DRAM shapes)
    W1 = np.asarray(W1, dtype=np.float32)
    W2 = np.asarray(W2, dtype=np.float32)
    b1 = np.asarray(b1, dtype=np.float32)
    b2 = np.asarray(b2, dtype=np.float32)
    w1h = np.ascontiguousarray(
        W1.reshape(E, DK, P, FT, P).transpose(0, 3, 2, 1, 4)
    ).astype(BF16)  # [E, FT, P(dp), DK, P(fi)]
    w2h = np.ascontiguousarray(W2.reshape(E, FT, P, D)).astype(BF16)  # [E, FT, P, D]
    b1h = np.ascontiguousarray(b1.reshape(E, FT, P).transpose(0, 2, 1))  # [E, P, FT]

    def pack(xg, c0, cn):
        blk = xg[c0 : c0 + cn].T.reshape(DK, P, cn).transpose(1, 0, 2)
        return blk.reshape(P, DK * cn).astype(BF16)

    in_maps = []
    for e in range(E):
        toks = token_lists[e]
        xg = np.zeros((C, d), dtype=np.float32)
        xg[: len(toks)] = xf[toks]
        xt = np.empty((P, DK * C), dtype=BF16)
        for c0, cn in _token_chunks(C):
            xt[:, c0 * DK : c0 * DK + DK * cn] = pack(xg, c0, cn)
        in_maps.append({"xt": xt, "w1": w1h[e], "w2": w2h[e], "b1": b1h[e]})

    res = run_bass_kernel_spmd(
        nc, in_maps, core_ids=list(range(N_CORES)), trace=trace
    )

    # Host combine: out[t] = sum_k rw[t,k] * (y_{e_k}(t) + b2[e_k])
    w_dense = np.zeros((T, E), dtype=np.float32)
    np.put_along_axis(w_dense, top2, rw, axis=1)
    out = w_dense @ b2  # [T, D] bias part
    for e in range(E):
        toks = token_lists[e]
        yr = np.asarray(res.results[e]["y"], dtype=np.float32)  # [TT//2, P, 2, D]
        y = yr.transpose(0, 2, 1, 3).reshape(C, d)
        out[toks] += weight_lists[e][:, None] * y[: len(toks)]

    # Host-side spill: overflow pairs beyond the device capacity.
    try:
        from scipy.special import erf
    except ImportError:
        import math

        erf = np.vectorize(math.erf, otypes=[np.float32])

    sqrt2 = np.float32(np.sqrt(2.0))
    for e in range(E):
        toks, w = spill_lists[e]
        if len(toks) == 0:
            continue
        hs = xf[toks] @ W1[e] + b1[e]
        hs = 0.5 * hs * (1.0 + erf(hs / sqrt2))
        ys = hs @ W2[e]
        out[toks] += w[:, None] * ys

    return out.reshape(B, S, d).astype(np.float32), res


def kernel(x, Wr, br, W1, b1, W2, b2):
    out, _ = _run(x, Wr, br, W1, b1, W2, b2, trace=False)
    return out



# revision 39
# speedup vs baseline: 1.0113x; 1.0113x over previous
"""MoE (top-2 of 8 experts) Trainium2 kernel, expert-parallel across 8 NeuronCores.

Strategy (matches the expert-parallel sharding hint):
  - Host computes the router (logits -> top-2 -> softmax) and performs the
    token all-to-all: tokens are gathered per expert, padded to a common
    capacity C, and each core gets one expert's tokens + that expert's
    W1/b1/W2 weights.
  - Each core runs a Bass/Tile kernel computing
        y = gelu_exact(x @ W1 + b1) @ W2
    in bf16 (fp32 PSUM accumulate, ~3e-3 rel err, well under the 2e-2 gate).
  - Host scatter-adds the per-expert outputs back with the routing weights
    and adds sum_k w_k * b2[e_k] (folding b2 into the host combine).

Per-core dataflow (two phases, PE never idles between them):
  Phase A (h = gelu(x @ W1 + b1)): stationary = W1 128x128 blocks streamed
  from HBM, moving = x token blocks, all N=512 wide so the 150ns LDWEIGHTS
  stays hidden under 216ns matmuls; PSUM [f, 512 tok]; exact GELU +
  per-partition bias b1 fused into one ScalarE activation per tile; h kept
  RESIDENT in SBUF as bf16. Startup is bandwidth-walled (~240GB/s/core
  while all 8 cores pull their first bytes), so the critical transfers are
  demand-ordered across the sync/gpsimd/scalar DMA queues: x block 0 rides
  both rings as two dk-half TILES (the first 4 matmuls start when the lo
  half lands), w1_0 on the scalar ring, then a W1 ladder alternating
  rings; the first K=10 f-tiles run block 0 only, then their deferred
  block-1 groups (W1 held resident, W2 prefetch pulled into this no-new-
  data window), so x block 1's deadline sits past the bandwidth wall.
  Phase B (y = h @ W2): W2 fully resident in one SBUF tile (prefetched
  behind the W1 stream); stationary = h blocks [128 f, 128 tok], moving =
  W2 rows [128 f, 512 d]; each token pair's y accumulates over all 32
  f-tiles in dedicated PSUM banks, then drains (ScalarE+VectorE halves in
  parallel -> bf16 -> DMA) while the next pair accumulates; the last tile
  runs as three pieces (512/256/256 d-cols) in separate recycled PSUM
  tiles so only a 64KB drain+DMA sits in the program tail.
"""

import numpy as np
import ml_dtypes

import concourse.mybir as mybir
import concourse.tile as tile
from concourse import bacc
from concourse.bass_utils import run_bass_kernel_spmd

P = 128
D = 1024
F = 4096
E = 8
TOP_K = 2
DK = D // P   # 8 contraction tiles for GEMM1
FT = F // P   # 32 f tiles
N_CORES = 8

BF16 = ml_dtypes.bfloat16

_F32 = mybir.dt.float32
_BF16 = mybir.dt.bfloat16

_compiled = {}  # C -> Bacc program


def _token_chunks(C):
    """Split C into 512-token chunks (PSUM-bank-width moving dim)."""
    chunks = []
    off = 0
    while off < C:
        cn = min(512, C - off)
        chunks.append((off, cn))
        off += cn
    return chunks


def _build(C):
    assert C % 256 == 0
    TT = C // P   # token tiles for GEMM2
    blocks = _token_chunks(C)   # 512-token blocks: h layout / phase B
    K = min(10, FT)             # f-tiles that run before x block 1 arrives
    nc = bacc.Bacc(None, target_bir_lowering=False)

    # x layout: dk-major 512-token blocks ([P, DK, 512] each, contiguous
    # per block so startup DMAs can slice dk-halves of block 0).
    xt_d = nc.dram_tensor("xt", [P, DK * C], _BF16, kind="ExternalInput")
    w1_d = nc.dram_tensor("w1", [FT, P, DK, P], _BF16, kind="ExternalInput")
    w2_d = nc.dram_tensor("w2", [FT, P, D], _BF16, kind="ExternalInput")
    b1_d = nc.dram_tensor("b1", [P, FT], _F32, kind="ExternalInput")
    y_d = nc.dram_tensor("y", [TT // 2, P, 2, D], _BF16, kind="ExternalOutput")

    with tile.TileContext(nc) as tc:
        with (
            tc.tile_pool(name="xpool", bufs=1) as xpool,
            tc.tile_pool(name="cpool", bufs=1) as cpool,
            tc.tile_pool(name="w1pool", bufs=1) as w1pool,
            tc.tile_pool(name="w2pool", bufs=1) as w2pool,
            tc.tile_pool(name="hpool", bufs=1) as hpool,
            tc.tile_pool(name="ypool", bufs=2) as ypool,
            # 4 GEMM1 banks absorb ScalarE GELU / supply jitter three groups
            # deep; the sequential phase B reuses an acc tile only every
            # ~27us, so 2 ypsum tiles (4 banks) suffice. 4 + 4 = all 8 banks.
            tc.tile_pool(name="hpsum", bufs=4, space="PSUM") as hpsum,
            tc.tile_pool(name="ypsum", bufs=2, space="PSUM") as ypsum,
        ):
            # All inputs ride the sync DMA ring in demand order (the ring is
            # FIFO; emission order is preserved for these uniform triggers).
            # gate() pins a trigger behind the first x chunk via a WAW write
            # into its destination, so the list-scheduler cannot hoist it.
            def gate(dst_corner, src_tile):
                nc.vector.tensor_copy(dst_corner, src_tile[:, 0, 0:2])

            w1_live = K + 2  # first K held through their deferred groups
            solo = {}

            def w1_dma(ft, gated=False, eng=None, gate_src=None, split=False):
                t = w1pool.tile(
                    [P, DK, P], _BF16, tag="w1t", bufs=w1_live, name=f"w1s{ft}"
                )
                if gated or gate_src is not None:
                    gate(t[:, 0, 0:2], gate_src if gate_src is not None else xt_sb[0])
                if split:
                    # startup-critical: halves ride both DMA rings in parallel
                    h = DK // 2
                    nc.sync.dma_start(out=t[:, 0:h], in_=w1_d[ft][:, 0:h])
                    nc.gpsimd.dma_start(out=t[:, h:DK], in_=w1_d[ft][:, h:DK])
                else:
                    (eng or nc.sync).dma_start(out=t[:], in_=w1_d[ft])
                solo[ft] = t

            w2_sb = w2pool.tile([P, FT, D], _BF16, name="w2sb")
            w2_fill = [0]

            def w2_dma(gated=False):
                k = w2_fill[0]
                if k < FT:
                    if gated:
                        gate(w2_sb[:, k, 0:2], xt_sb[0])
                    nc.sync.dma_start(out=w2_sb[:, k], in_=w2_d[k])
                    w2_fill[0] = k + 1

            h_sb = [
                hpool.tile([P, FT, cn], _BF16, tag=f"hc{ci}", name=f"hc{ci}")
                for ci, (_, cn) in enumerate(blocks)
            ]

            # chunk list for GEMM1: (x-dram-offset, width, h-offset).
            # All chunks are full 512-token blocks: ring-split startup DMA
            # gets block 0 on-chip by ~11.5us, and N=512 keeps every group's
            # LDWEIGHTS hidden under 216ns matmuls (256-wide groups are
            # LDWEIGHTS-bound: 151ns load > 109ns matmul).
            xchunks = [(DK * c0, cn, c0) for c0, cn in blocks]
            xt_sb = []
            x0_hi = [None]  # dk 4..7 of block 0 (separate tile: see x_dma)

            def x_dma(si, gated=False, split=False, dk_tiles=False):
                o, cn, _ = xchunks[si]
                if dk_tiles:
                    # Block 0 as TWO dk-half tiles so the first group's
                    # dk 0..3 matmuls start when the lo half lands (~2.3us
                    # before the full block would; tile-granular dependency
                    # tracking would otherwise hold them for all of x0).
                    # Each half still ring-splits for parallel transfer.
                    hw = DK // 2 * cn
                    lo = xpool.tile([P, DK // 2, cn], _BF16, tag="xt0lo", name="xt0lo")
                    hi = xpool.tile([P, DK // 2, cn], _BF16, tag="xt0hi", name="xt0hi")
                    qw = hw // 2
                    nc.sync.dma_start(out=lo[:, 0 : DK // 4], in_=xt_d[:, o : o + qw])
                    nc.gpsimd.dma_start(
                        out=lo[:, DK // 4 :], in_=xt_d[:, o + qw : o + hw]
                    )
                    nc.sync.dma_start(
                        out=hi[:, 0 : DK // 4], in_=xt_d[:, o + hw : o + hw + qw]
                    )
                    nc.gpsimd.dma_start(
                        out=hi[:, DK // 4 :], in_=xt_d[:, o + hw + qw : o + 2 * hw]
                    )
                    xt_sb.append(lo)
                    x0_hi[0] = hi
                    return
                t = xpool.tile([P, DK, cn], _BF16, tag=f"xt{si}", name=f"xt{si}")
                if gated:
                    gate(t[:, 0, 0:2], xt_sb[0])
                if split:
                    # startup-critical: halves ride both DMA rings in parallel
                    hw = DK // 2 * cn
                    nc.sync.dma_start(out=t[:, 0 : DK // 2], in_=xt_d[:, o : o + hw])
                    nc.gpsimd.dma_start(
                        out=t[:, DK // 2 : DK], in_=xt_d[:, o + hw : o + 2 * hw]
                    )
                else:
                    nc.sync.dma_start(out=t[:], in_=xt_d[:, o : o + DK * cn])
                xt_sb.append(t)

            def x_mov(si, dk):
                """Moving operand for (chunk si, contraction tile dk)."""
                if si == 0 and x0_hi[0] is not None:
                    t = xt_sb[0] if dk < DK // 2 else x0_hi[0]
                    return t[:, dk % (DK // 2), :]
                return xt_sb[si][:, dk, :]

            # Startup emission. Each DMA trigger costs ~0.6-0.7us of QUEUE
            # time, so the critical first wave is spread across FOUR queues
            # (scalar + vector are idle at startup and can trigger DMAs too):
            #   sync:   x0 lo-half      gpsimd: x0 hi-half
            #   scalar: w1_0 (then the dummy-GELU table load)
            #   vector: b1, w1_1 (then the warm memset + gates)
            # The first group's inputs (x0+w1_0+b1 = 1.28MB) then complete
            # at the aggregate-bandwidth floor (~11.5us); later tiles queue
            # FIFO behind them on the two rings.
            # Warm-tile memset FIRST on the gpsimd queue (runs ~6.2us, before
            # its DMA trigger) so the PE warm-up starts during engine init.
            warm = cpool.tile([P, 512], _BF16, tag="warm")
            nc.gpsimd.memset(warm[:], 0.0)
            w1_dma(0, eng=nc.scalar)
            b1_sb = cpool.tile([P, FT], _F32)
            nc.scalar.dma_start(out=b1_sb[:], in_=b1_d[:])
            x_dma(0, dk_tiles=True)
            # W1 ladder BEHIND x0's halves: w1_1 is split across both rings
            # (it is due only ~1.7us after x0 lands, sooner than a whole
            # tile can follow x0 on one ring); the rest alternate rings,
            # landing every ~1.1us (two rings at ~120GB/s each under the
            # 8-core startup contention) vs the 1.73us/tile consumption.
            # K=10 pushes x block 1's deadline past the ~240GB/s wall.
            w1_dma(1, split=True)
            for ft in range(2, K):
                w1_dma(ft, eng=nc.sync if ft % 2 == 0 else nc.gpsimd)
            # Dummy GELU on scratch: pulls ScalarE's ~1.3us ACT_TABLE_LOAD
            # into the startup DMA wait (scalar queue: w1_0 trigger, then
            # this). Otherwise the FIRST real GELU pays it, holds an hpsum
            # buffer longer, and stalls the PE through the rotation.
            scratch = cpool.tile([P, 16], _BF16, tag="scr")
            nc.scalar.activation(
                scratch[:],
                warm[:, 0:16],
                mybir.ActivationFunctionType.Gelu,
                bias=warm[:, 0:1],
                scale=1.0,
            )
            # PE warm-up: dummy zero matmuls with no DMA deps run during the
            # initial input-DMA wait, so the HAM clock gate reaches 2.4 GHz
            # before the real stream starts; sized to end at the measured
            # block-0 arrival (~11.5us).
            # Sized to cover the SLOWEST core's data arrival (~13us): the
            # max-core sets the graded time, and an idle gap before its
            # first real matmul also resets the HAM clock ramp (~2us of
            # 379ns mid-pstate matmuls). Fast cores just queue briefly.
            WARM = (9, 8)
            for r, nw in enumerate(WARM):
                pw = hpsum.tile([P, 512], _F32, tag="ph", name=f"pw{r}")
                for k in range(nw):
                    nc.tensor.matmul(
                        pw[:], warm[:, :P], warm[:], start=(k == 0), stop=(k == nw - 1)
                    )
            # x block 1 follows the W1 ladder on both rings (ring FIFO
            # orders the transfers; the gate pins emission order against
            # list-scheduler hoisting).
            if len(xchunks) > 1:
                x_dma(1, split=True, gated=True)

            def gemm1_group(ft, si):
                _, cn, h0 = xchunks[si]
                ph = hpsum.tile([P, 512], _F32, tag="ph")
                for dk in range(DK):
                    nc.tensor.matmul(
                        ph[:, :cn],
                        solo[ft][:, dk, :],
                        x_mov(si, dk),
                        start=(dk == 0),
                        stop=(dk == DK - 1),
                    )
                nc.scalar.activation(
                    h_sb[h0 // 512][:, ft, h0 % 512 : h0 % 512 + cn],
                    ph[:, :cn],
                    mybir.ActivationFunctionType.Gelu,
                    bias=b1_sb[:, ft : ft + 1],
                    scale=1.0,
                )

            # Phase A order: the first K f-tiles run block 0 while block 1
            # streams in, then their deferred block-1 groups (W1 tiles held
            # resident); the rest run f-tile-major over both blocks so each
            # W1 tile is streamed exactly once.
            NB = len(xchunks)
            order = [(ft, 0) for ft in range(K)]
            for b in range(1, NB):
                order += [(ft, b) for ft in range(K)]
            order += [(ft, b) for ft in range(K, FT) for b in range(NB)]

            seen = set(ft for ft, _ in order[:K])
            for ft, si in order:
                if ft not in solo:
                    w1_dma(ft, gated=True)
                if ft not in seen:
                    seen.add(ft)
                    # W1 lookahead + W2 prefetch ride the same ring.
                    la = ft + 2
                    if la < FT and la not in solo:
                        w1_dma(la, gated=True)
                    w2_dma(gated=w2_fill[0] < 2)
                    w2_dma(gated=w2_fill[0] < 2)
                elif si >= 1 and ft < K:
                    # deferred-block groups consume no new data: use their
                    # window to pull the W2 prefetch forward
                    w2_dma(gated=w2_fill[0] < 2)
                gemm1_group(ft, si)
            while w2_fill[0] < FT:
                w2_dma()

            # Phase B: token tiles, full 32-step PSUM accumulation each.
            # The two tiles of a pair run SEQUENTIALLY (not ft-interleaved),
            # so each tile's drain + y DMA overlaps the next tile's 64-MM
            # accumulation; only the very last tile's drain+DMA lands in the
            # program tail.
            for tq in range(TT // 2):
                ci = (tq * 2 * P) // 512  # block holding this token pair
                cb = tq * 2 * P - blocks[ci][0]  # base token within block
                ysb = ypool.tile([P, 2, D], _BF16, tag="ysb")
                for tt2 in range(2):
                    acc = ypsum.tile([P, D], _F32, tag="py", name=f"py{tq}_{tt2}")
                    eng = nc.sync if (tq * 2 + tt2) % 2 == 0 else nc.gpsimd
                    if tq == TT // 2 - 1 and tt2 == 1:
                        # Final token tile: d-half 0, then two d-quarters as
                        # separate sequential accumulation groups, so the
                        # program-tail drain + y DMA is only 64KB (the drains
                        # of the earlier pieces hide under later matmuls).
                        # Each piece gets its OWN PSUM tile (recycled from the
                        # idle phase-A pool): slicing one shared acc tile puts
                        # a false WAR between piece N's drain-read and piece
                        # N+1's first matmul (~0.8us PE stall each, measured).
                        pieces = [(0, 512), (512, 256), (768, 256)]
                        for pi, (d0, dn) in enumerate(pieces):
                            pt = hpsum.tile([P, 512], _F32, tag="ph", name=f"pyf{pi}")
                            for ft in range(FT):
                                hblk = h_sb[ci][
                                    :, ft, cb + tt2 * P : cb + (tt2 + 1) * P
                                ]
                                nc.tensor.matmul(
                                    pt[:, :dn],
                                    hblk,
                                    w2_sb[:, ft, d0 : d0 + dn],
                                    start=(ft == 0),
                                    stop=(ft == FT - 1),
                                )
                            if pi == len(pieces) - 1:
                                # Last piece: drain + ship as two parallel
                                # halves (Scalar+sync / Vector+gpsimd) to
                                # halve the program-tail serial chain.
                                hn = dn // 2
                                nc.scalar.activation(
                                    ysb[:, tt2, d0 : d0 + hn],
                                    pt[:, :hn],
                                    mybir.ActivationFunctionType.Copy,
                                )
                                nc.sync.dma_start(
                                    out=y_d[tq, :, tt2, d0 : d0 + hn],
                                    in_=ysb[:, tt2, d0 : d0 + hn],
                                )
                                nc.vector.tensor_copy(
                                    ysb[:, tt2, d0 + hn : d0 + dn],
                                    pt[:, hn:dn],
                                )
                                nc.gpsimd.dma_start(
                                    out=y_d[tq, :, tt2, d0 + hn : d0 + dn],
                                    in_=ysb[:, tt2, d0 + hn : d0 + dn],
                                )
                            elif pi % 2 == 0:
                                nc.scalar.activation(
                                    ysb[:, tt2, d0 : d0 + dn],
                                    pt[:, :dn],
                                    mybir.ActivationFunctionType.Copy,
                                )
                                nc.sync.dma_start(
                                    out=y_d[tq, :, tt2, d0 : d0 + dn],
                                    in_=ysb[:, tt2, d0 : d0 + dn],
                                )
                            else:
                                nc.vector.tensor_copy(
                                    ysb[:, tt2, d0 : d0 + dn],
                                    pt[:, :dn],
                                )
                                nc.gpsimd.dma_start(
                                    out=y_d[tq, :, tt2, d0 : d0 + dn],
                                    in_=ysb[:, tt2, d0 : d0 + dn],
                                )
                        continue
                    for ft in range(FT):
                        hblk = h_sb[ci][:, ft, cb + tt2 * P : cb + (tt2 + 1) * P]
                        for dh in range(2):
                            nc.tensor.matmul(
                                acc[:, dh * 512 : (dh + 1) * 512],
                                hblk,
                                w2_sb[:, ft, dh * 512 : (dh + 1) * 512],
                                start=(ft == 0),
                                stop=(ft == FT - 1),
                            )
                    # Drain the two PSUM banks in parallel on Scalar+Vector,
                    # then ship this token tile immediately.
                    nc.scalar.activation(
                        ysb[:, tt2, :512],
                        acc[:, :512],
                        mybir.ActivationFunctionType.Copy,
                    )
                    nc.vector.tensor_copy(ysb[:, tt2, 512:], acc[:, 512:])
                    eng.dma_start(out=y_d[tq, :, tt2, :], in_=ysb[:, tt2, :])

    nc.compile()
    return nc


def _route(xf, Wr, br):
    """Host router: exact top-2 + softmax weights (float64 for stable order)."""
    logits = xf.astype(np.float64) @ Wr.astype(np.float64) + br.astype(np.float64)
    order = np.argsort(-logits, axis=1, kind="stable")
    top2 = order[:, :TOP_K]  # [T, 2]
    v = np.take_along_axis(logits, top2, axis=1)
    v = v - v.max(axis=1, keepdims=True)
    ev = np.exp(v)
    rw = (ev / ev.sum(axis=1, keepdims=True)).astype(np.float32)  # [T, 2]
    return top2, rw


def _run(x, Wr, br, W1, b1, W2, b2, trace=False):
    B, S, d = x.shape
    T = B * S
    xf = np.ascontiguousarray(np.asarray(x, dtype=np.float32).reshape(T, d))

    top2, rw = _route(xf, Wr, br)

    token_lists = []
    weight_lists = []
    for e in range(E):
        in_slot0 = top2[:, 0] == e
        in_slot1 = top2[:, 1] == e
        toks = np.nonzero(in_slot0 | in_slot1)[0]
        w = np.where(in_slot0[toks], rw[toks, 0], rw[toks, 1]).astype(np.float32)
        token_lists.append(toks)
        weight_lists.append(w)

    # Capacity: balanced mean (rounded up to 256), capped by the SBUF
    # working set (x + h + W2 are resident). Pairs beyond it are computed
    # on the host - cheap for near-balanced routing.
    C = max(256, min(1024, -(-(T * TOP_K // E) // 256) * 256))
    spill_lists = [(t[C:], w[C:]) for t, w in zip(token_lists, weight_lists)]
    token_lists = [t[:C] for t in token_lists]
    weight_lists = [w[:C] for w in weight_lists]

    if C not in _compiled:
        _compiled[C] = _build(C)
    nc = _compiled[C]

    # Per-expert weight layouts (see _build DRAM shapes)
    W1 = np.asarray(W1, dtype=np.float32)
    W2 = np.asarray(W2, dtype=np.float32)
    b1 = np.asarray(b1, dtype=np.float32)
    b2 = np.asarray(b2, dtype=np.float32)
    w1h = np.ascontiguousarray(
        W1.reshape(E, DK, P, FT, P).transpose(0, 3, 2, 1, 4)
    ).astype(BF16)  # [E, FT, P(dp), DK, P(fi)]
    w2h = np.ascontiguousarray(W2.reshape(E, FT, P, D)).astype(BF16)  # [E, FT, P, D]
    b1h = np.ascontiguousarray(b1.reshape(E, FT, P).transpose(0, 2, 1))  # [E, P, FT]

    def pack(xg, c0, cn):
        blk = xg[c0 : c0 + cn].T.reshape(DK, P, cn).transpose(1, 0, 2)
        return blk.reshape(P, DK * cn).astype(BF16)

    in_maps = []
    for e in range(E):
        toks = token_lists[e]
        xg = np.zeros((C, d), dtype=np.float32)
        xg[: len(toks)] = xf[toks]
        xt = np.empty((P, DK * C), dtype=BF16)
        for c0, cn in _token_chunks(C):
            xt[:, c0 * DK : c0 * DK + DK * cn] = pack(xg, c0, cn)
        in_maps.append({"xt": xt, "w1": w1h[e], "w2": w2h[e], "b1": b1h[e]})

    res = run_bass_kernel_spmd(
        nc, in_maps, core_ids=list(range(N_CORES)), trace=trace
    )

    # Host combine: out[t] = sum_k rw[t,k] * (y_{e_k}(t) + b2[e_k])
    w_dense = np.zeros((T, E), dtype=np.float32)
    np.put_along_axis(w_dense, top2, rw, axis=1)
    out = w_dense @ b2  # [T, D] bias part
    for e in range(E):
        toks = token_lists[e]
        yr = np.asarray(res.results[e]["y"], dtype=np.float32)  # [TT//2, P, 2, D]
        y = yr.transpose(0, 2, 1, 3).reshape(C, d)
        out[toks] += weight_lists[e][:, None] * y[: len(toks)]

    # Host-side spill: overflow pairs beyond the device capacity.
    try:
        from scipy.special import erf
    except ImportError:
        import math

        erf = np.vectorize(math.erf, otypes=[np.float32])

    sqrt2 = np.float32(np.sqrt(2.0))
    for e in range(E):
        toks, w = spill_lists[e]
        if len(toks) == 0:
            continue
        hs = xf[toks] @ W1[e] + b1[e]
        hs = 0.5 * hs * (1.0 + erf(hs / sqrt2))
        ys = hs @ W2[e]
        out[toks] += w[:, None] * ys

    return out.reshape(B, S, d).astype(np.float32), res


def kernel(x, Wr, br, W1, b1, W2, b2):
    out, _ = _run(x, Wr, br, W1, b1, W2, b2, trace=False)
    return out



# revision 45
# speedup vs baseline: 1.0184x; 1.0071x over previous
"""MoE (top-2 of 8 experts) Trainium2 kernel, expert-parallel across 8 NeuronCores.

Strategy (matches the expert-parallel sharding hint):
  - Host computes the router (logits -> top-2 -> softmax) and performs the
    token all-to-all: tokens are gathered per expert, padded to a common
    capacity C, and each core gets one expert's tokens + that expert's
    W1/b1/W2 weights.
  - Each core runs a Bass/Tile kernel computing
        y = gelu_exact(x @ W1 + b1) @ W2
    in bf16 (fp32 PSUM accumulate, ~3e-3 rel err, well under the 2e-2 gate).
  - Host scatter-adds the per-expert outputs back with the routing weights
    and adds sum_k w_k * b2[e_k] (folding b2 into the host combine).

Per-core dataflow (two phases, PE never idles between them):
  Phase A (h = gelu(x @ W1 + b1)): stationary = W1 128x128 blocks streamed
  from HBM, moving = x token blocks, all N=512 wide so the 150ns LDWEIGHTS
  stays hidden under 216ns matmuls; PSUM [f, 512 tok]; exact GELU +
  per-partition bias b1 fused into one ScalarE activation per tile; h kept
  RESIDENT in SBUF as bf16. Startup is bandwidth-walled (~240GB/s/core
  while all 8 cores pull their first bytes), so the critical transfers are
  demand-ordered across the sync/gpsimd/scalar DMA queues: x block 0 rides
  both rings as two dk-half TILES (the first 4 matmuls start when the lo
  half lands), w1_0 on the scalar ring, then a W1 ladder alternating
  rings; the first K=10 f-tiles run block 0 only, then their deferred
  block-1 groups (W1 held resident, W2 prefetch pulled into this no-new-
  data window), so x block 1's deadline sits past the bandwidth wall.
  Phase B (y = h @ W2): W2 fully resident in one SBUF tile (prefetched
  behind the W1 stream); stationary = h blocks [128 f, 128 tok], moving =
  W2 rows [128 f, 512 d]; each token pair's y accumulates over all 32
  f-tiles in dedicated PSUM banks, then drains (ScalarE+VectorE halves in
  parallel -> bf16 -> DMA) while the next pair accumulates; the last tile
  runs as three pieces (512/256/256 d-cols) in separate recycled PSUM
  tiles so only a 64KB drain+DMA sits in the program tail.
"""

import numpy as np
import ml_dtypes

import concourse.mybir as mybir
import concourse.tile as tile
from concourse import bacc
from concourse.bass_utils import run_bass_kernel_spmd

P = 128
D = 1024
F = 4096
E = 8
TOP_K = 2
DK = D // P   # 8 contraction tiles for GEMM1
FT = F // P   # 32 f tiles
N_CORES = 8

BF16 = ml_dtypes.bfloat16

_F32 = mybir.dt.float32
_BF16 = mybir.dt.bfloat16
_FP8 = mybir.dt.float8e4
FP8 = ml_dtypes.float8_e4m3

_compiled = {}  # C -> Bacc program


def _token_chunks(C):
    """Split C into 512-token chunks (PSUM-bank-width moving dim)."""
    chunks = []
    off = 0
    while off < C:
        cn = min(512, C - off)
        chunks.append((off, cn))
        off += cn
    return chunks


def _build(C):
    assert C % 256 == 0
    TT = C // P   # token tiles for GEMM2
    blocks = _token_chunks(C)   # 512-token blocks: h layout / phase B
    K = min(10, FT)             # f-tiles that run before x block 1 arrives
    nc = bacc.Bacc(None, target_bir_lowering=False)

    # x layout: dk-major 512-token blocks ([P, DK, 512] each, contiguous
    # per block so startup DMAs can slice dk-halves). x8 is an fp8e4m3 copy
    # of block 0 (half the bytes through the startup bandwidth wall): the
    # first K groups run stationary-bf16 x moving-fp8 matmuls (measured
    # exact vs cast on HW); ft>=K groups use the bf16 re-ship that arrives
    # off the critical path. fp8 x-quantization (~3.6% RMS/elem) on K/64
    # of h costs ~1e-2 final rel_l2 vs the 2e-2 gate.
    xt_d = nc.dram_tensor("xt", [P, DK * C], _BF16, kind="ExternalInput")
    cn0 = blocks[0][1]
    x8_d = nc.dram_tensor("x8", [P, DK * cn0], _FP8, kind="ExternalInput")
    w1_d = nc.dram_tensor("w1", [FT, P, DK, P], _BF16, kind="ExternalInput")
    w2_d = nc.dram_tensor("w2", [FT, P, D], _BF16, kind="ExternalInput")
    b1_d = nc.dram_tensor("b1", [P, FT], _F32, kind="ExternalInput")
    y_d = nc.dram_tensor("y", [TT // 2, P, 2, D], _BF16, kind="ExternalOutput")

    with tile.TileContext(nc) as tc:
        with (
            tc.tile_pool(name="xpool", bufs=1) as xpool,
            tc.tile_pool(name="cpool", bufs=1) as cpool,
            tc.tile_pool(name="w1pool", bufs=1) as w1pool,
            tc.tile_pool(name="w2pool", bufs=1) as w2pool,
            tc.tile_pool(name="hpool", bufs=1) as hpool,
            tc.tile_pool(name="ypool", bufs=2) as ypool,
            # 4 GEMM1 banks absorb ScalarE GELU / supply jitter three groups
            # deep; the sequential phase B reuses an acc tile only every
            # ~27us, so 2 ypsum tiles (4 banks) suffice. 4 + 4 = all 8 banks.
            tc.tile_pool(name="hpsum", bufs=4, space="PSUM") as hpsum,
            tc.tile_pool(name="ypsum", bufs=2, space="PSUM") as ypsum,
        ):
            # All inputs ride the sync DMA ring in demand order (the ring is
            # FIFO; emission order is preserved for these uniform triggers).
            # gate() pins a trigger behind the first x chunk via a WAW write
            # into its destination, so the list-scheduler cannot hoist it.
            def gate(dst_corner, src_tile):
                nc.vector.tensor_copy(dst_corner, src_tile[:, 0, 0:2])

            w1_live = K + 2  # first K held through their deferred groups
            solo = {}

            def w1_dma(ft, gated=False, eng=None, gate_src=None, split=False):
                t = w1pool.tile(
                    [P, DK, P], _BF16, tag="w1t", bufs=w1_live, name=f"w1s{ft}"
                )
                if gated or gate_src is not None:
                    gate(t[:, 0, 0:2], gate_src if gate_src is not None else x8t[0])
                if split:
                    # startup-critical: halves ride both DMA rings in parallel
                    h = DK // 2
                    nc.sync.dma_start(out=t[:, 0:h], in_=w1_d[ft][:, 0:h])
                    nc.gpsimd.dma_start(out=t[:, h:DK], in_=w1_d[ft][:, h:DK])
                else:
                    (eng or nc.sync).dma_start(out=t[:], in_=w1_d[ft])
                solo[ft] = t

            w2_sb = w2pool.tile([P, FT, D], _BF16, name="w2sb")
            w2_fill = [0]

            def w2_dma(gated=False):
                k = w2_fill[0]
                if k < FT:
                    if gated:
                        gate(w2_sb[:, k, 0:2], x8t[0])
                    nc.sync.dma_start(out=w2_sb[:, k], in_=w2_d[k])
                    w2_fill[0] = k + 1

            h_sb = [
                hpool.tile([P, FT, cn], _BF16, tag=f"hc{ci}", name=f"hc{ci}")
                for ci, (_, cn) in enumerate(blocks)
            ]

            # chunk list for GEMM1: (x-dram-offset, width, h-offset).
            # All chunks are full 512-token blocks: ring-split startup DMA
            # gets block 0 on-chip by ~11.5us, and N=512 keeps every group's
            # LDWEIGHTS hidden under 216ns matmuls (256-wide groups are
            # LDWEIGHTS-bound: 151ns load > 109ns matmul).
            xchunks = [(DK * c0, cn, c0) for c0, cn in blocks]
            xt_sb = {}
            x8t = []  # fp8 dk-half tiles of block 0

            def x8_dma():
                # Block 0 in fp8 as TWO dk-half tiles, each ring-split: the
                # first group's dk 0..3 matmuls start when the lo half (a
                # quarter of the bf16 block's bytes) lands; tile-granular
                # dependency tracking would otherwise hold them for all of
                # x0.
                hw = DK // 2 * cn0
                qw = hw // 2
                lo = xpool.tile([P, DK // 2, cn0], _FP8, tag="x8lo", name="x8lo")
                hi = xpool.tile([P, DK // 2, cn0], _FP8, tag="x8hi", name="x8hi")
                nc.sync.dma_start(out=lo[:, 0 : DK // 4], in_=x8_d[:, 0:qw])
                nc.gpsimd.dma_start(out=lo[:, DK // 4 :], in_=x8_d[:, qw:hw])
                nc.sync.dma_start(out=hi[:, 0 : DK // 4], in_=x8_d[:, hw : hw + qw])
                nc.gpsimd.dma_start(out=hi[:, DK // 4 :], in_=x8_d[:, hw + qw :])
                x8t.extend([lo, hi])

            def x_dma(si, gated=False, split=False):
                o, cn, _ = xchunks[si]
                t = xpool.tile([P, DK, cn], _BF16, tag=f"xt{si}", name=f"xt{si}")
                if gated:
                    gate(t[:, 0, 0:2], x8t[0])
                if split:
                    # halves ride both DMA rings in parallel
                    hw = DK // 2 * cn
                    nc.sync.dma_start(out=t[:, 0 : DK // 2], in_=xt_d[:, o : o + hw])
                    nc.gpsimd.dma_start(
                        out=t[:, DK // 2 : DK], in_=xt_d[:, o + hw : o + 2 * hw]
                    )
                else:
                    nc.sync.dma_start(out=t[:], in_=xt_d[:, o : o + DK * cn])
                xt_sb[si] = t

            def x_mov(si, dk, ft):
                """Moving operand for (chunk si, contraction tile dk)."""
                if si == 0 and ft < K:
                    t = x8t[0] if dk < DK // 2 else x8t[1]
                    return t[:, dk % (DK // 2), :]
                return xt_sb[si][:, dk, :]

            # Startup emission. Each DMA trigger costs ~0.6-0.7us of QUEUE
            # time, so the critical first wave is spread across FOUR queues
            # (scalar + vector are idle at startup and can trigger DMAs too):
            #   sync:   x0 lo-half      gpsimd: x0 hi-half
            #   scalar: w1_0 (then the dummy-GELU table load)
            #   vector: b1, w1_1 (then the warm memset + gates)
            # The first group's inputs (x0+w1_0+b1 = 1.28MB) then complete
            # at the aggregate-bandwidth floor (~11.5us); later tiles queue
            # FIFO behind them on the two rings.
            # Warm-tile memset FIRST on the gpsimd queue (runs ~6.2us, before
            # its DMA trigger) so the PE warm-up starts during engine init.
            warm = cpool.tile([P, 512], _BF16, tag="warm")
            nc.gpsimd.memset(warm[:], 0.0)
            w1_dma(0, eng=nc.scalar)
            b1_sb = cpool.tile([P, FT], _F32)
            nc.scalar.dma_start(out=b1_sb[:], in_=b1_d[:])
            x8_dma()
            # W1 ladder BEHIND x0's halves: w1_1 is split across both rings
            # (it is due only ~1.7us after x0 lands, sooner than a whole
            # tile can follow x0 on one ring); the rest alternate rings,
            # landing every ~1.1us (two rings at ~120GB/s each under the
            # 8-core startup contention) vs the 1.73us/tile consumption.
            # K=10 pushes x block 1's deadline past the ~240GB/s wall.
            w1_dma(1, split=True)
            for ft in range(2, K):
                w1_dma(ft, eng=nc.sync if ft % 2 == 0 else nc.gpsimd)
            # Dummy GELU on scratch: pulls ScalarE's ~1.3us ACT_TABLE_LOAD
            # into the startup DMA wait (scalar queue: w1_0 trigger, then
            # this). Otherwise the FIRST real GELU pays it, holds an hpsum
            # buffer longer, and stalls the PE through the rotation.
            scratch = cpool.tile([P, 16], _BF16, tag="scr")
            nc.scalar.activation(
                scratch[:],
                warm[:, 0:16],
                mybir.ActivationFunctionType.Gelu,
                bias=warm[:, 0:1],
                scale=1.0,
            )
            # PE warm-up: dummy zero matmuls with no DMA deps run during the
            # initial input-DMA wait, so the HAM clock gate reaches 2.4 GHz
            # before the real stream starts; sized to end at the measured
            # block-0 arrival (~11.5us).
            # Sized to cover the SLOWEST core's data arrival (~13us): the
            # max-core sets the graded time, and an idle gap before its
            # first real matmul also resets the HAM clock ramp (~2us of
            # 379ns mid-pstate matmuls). Fast cores just queue briefly.
            WARM = (7, 6)
            for r, nw in enumerate(WARM):
                pw = hpsum.tile([P, 512], _F32, tag="ph", name=f"pw{r}")
                for k in range(nw):
                    nc.tensor.matmul(
                        pw[:], warm[:, :P], warm[:], start=(k == 0), stop=(k == nw - 1)
                    )
            # x block 1 follows the W1 ladder on both rings (ring FIFO
            # orders the transfers; the gate pins emission order against
            # list-scheduler hoisting).
            if len(xchunks) > 1:
                x_dma(1, split=True, gated=True)
            # bf16 re-ship of block 0 for the ft>=K groups (due ~17us after
            # the fp8 copy; rides behind x block 1 on both rings)
            x_dma(0, split=True, gated=True)

            def gemm1_group(ft, si):
                _, cn, h0 = xchunks[si]
                ph = hpsum.tile([P, 512], _F32, tag="ph")
                for dk in range(DK):
                    nc.tensor.matmul(
                        ph[:, :cn],
                        solo[ft][:, dk, :],
                        x_mov(si, dk, ft),
                        start=(dk == 0),
                        stop=(dk == DK - 1),
                    )
                nc.scalar.activation(
                    h_sb[h0 // 512][:, ft, h0 % 512 : h0 % 512 + cn],
                    ph[:, :cn],
                    mybir.ActivationFunctionType.Gelu,
                    bias=b1_sb[:, ft : ft + 1],
                    scale=1.0,
                )

            # Phase A order: the first K f-tiles run block 0 while block 1
            # streams in, then their deferred block-1 groups (W1 tiles held
            # resident); the rest run f-tile-major over both blocks so each
            # W1 tile is streamed exactly once.
            NB = len(xchunks)
            order = [(ft, 0) for ft in range(K)]
            for b in range(1, NB):
                order += [(ft, b) for ft in range(K)]
            order += [(ft, b) for ft in range(K, FT) for b in range(NB)]

            seen = set(ft for ft, _ in order[:K])
            for ft, si in order:
                if ft not in solo:
                    w1_dma(ft, gated=True)
                if ft not in seen:
                    seen.add(ft)
                    # W1 lookahead + W2 prefetch ride the same ring.
                    la = ft + 2
                    if la < FT and la not in solo:
                        w1_dma(la, gated=True)
                    w2_dma(gated=w2_fill[0] < 2)
                    w2_dma(gated=w2_fill[0] < 2)
                elif si >= 1 and ft < K:
                    # deferred-block groups consume no new data: use their
                    # window to pull the W2 prefetch forward
                    w2_dma(gated=w2_fill[0] < 2)
                gemm1_group(ft, si)
            while w2_fill[0] < FT:
                w2_dma()

            # Phase B: token tiles, full 32-step PSUM accumulation each.
            # The two tiles of a pair run SEQUENTIALLY (not ft-interleaved),
            # so each tile's drain + y DMA overlaps the next tile's 64-MM
            # accumulation; only the very last tile's drain+DMA lands in the
            # program tail.
            for tq in range(TT // 2):
                ci = (tq * 2 * P) // 512  # block holding this token pair
                cb = tq * 2 * P - blocks[ci][0]  # base token within block
                ysb = ypool.tile([P, 2, D], _BF16, tag="ysb")
                for tt2 in range(2):
                    acc = ypsum.tile([P, D], _F32, tag="py", name=f"py{tq}_{tt2}")
                    eng = nc.sync if (tq * 2 + tt2) % 2 == 0 else nc.gpsimd
                    if tq == TT // 2 - 1 and tt2 == 1:
                        # Final token tile: d-half 0, then two d-quarters as
                        # separate sequential accumulation groups, so the
                        # program-tail drain + y DMA is only 64KB (the drains
                        # of the earlier pieces hide under later matmuls).
                        # Each piece gets its OWN PSUM tile (recycled from the
                        # idle phase-A pool): slicing one shared acc tile puts
                        # a false WAR between piece N's drain-read and piece
                        # N+1's first matmul (~0.8us PE stall each, measured).
                        pieces = [(0, 512), (512, 256), (768, 256)]
                        for pi, (d0, dn) in enumerate(pieces):
                            pt = hpsum.tile([P, 512], _F32, tag="ph", name=f"pyf{pi}")
                            for ft in range(FT):
                                hblk = h_sb[ci][
                                    :, ft, cb + tt2 * P : cb + (tt2 + 1) * P
                                ]
                                nc.tensor.matmul(
                                    pt[:, :dn],
                                    hblk,
                                    w2_sb[:, ft, d0 : d0 + dn],
                                    start=(ft == 0),
                                    stop=(ft == FT - 1),
                                )
                            if pi == len(pieces) - 1:
                                # Last piece: drain + ship as two parallel
                                # halves (Scalar+sync / Vector+gpsimd) to
                                # halve the program-tail serial chain.
                                hn = dn // 2
                                nc.scalar.activation(
                                    ysb[:, tt2, d0 : d0 + hn],
                                    pt[:, :hn],
                                    mybir.ActivationFunctionType.Copy,
                                )
                                nc.sync.dma_start(
                                    out=y_d[tq, :, tt2, d0 : d0 + hn],
                                    in_=ysb[:, tt2, d0 : d0 + hn],
                                )
                                nc.vector.tensor_copy(
                                    ysb[:, tt2, d0 + hn : d0 + dn],
                                    pt[:, hn:dn],
                                )
                                nc.gpsimd.dma_start(
                                    out=y_d[tq, :, tt2, d0 + hn : d0 + dn],
                                    in_=ysb[:, tt2, d0 + hn : d0 + dn],
                                )
                            elif pi % 2 == 0:
                                nc.scalar.activation(
                                    ysb[:, tt2, d0 : d0 + dn],
                                    pt[:, :dn],
                                    mybir.ActivationFunctionType.Copy,
                                )
                                nc.sync.dma_start(
                                    out=y_d[tq, :, tt2, d0 : d0 + dn],
                                    in_=ysb[:, tt2, d0 : d0 + dn],
                                )
                            else:
                                nc.vector.tensor_copy(
                                    ysb[:, tt2, d0 : d0 + dn],
                                    pt[:, :dn],
                                )
                                nc.gpsimd.dma_start(
                                    out=y_d[tq, :, tt2, d0 : d0 + dn],
                                    in_=ysb[:, tt2, d0 : d0 + dn],
                                )
                        continue
                    for ft in range(FT):
                        hblk = h_sb[ci][:, ft, cb + tt2 * P : cb + (tt2 + 1) * P]
                        for dh in range(2):
                            nc.tensor.matmul(
                                acc[:, dh * 512 : (dh + 1) * 512],
                                hblk,
                                w2_sb[:, ft, dh * 512 : (dh + 1) * 512],
                                start=(ft == 0),
                                stop=(ft == FT - 1),
                            )
                    # Drain the two PSUM banks in parallel on Scalar+Vector,
                    # then ship this token tile immediately.
                    nc.scalar.activation(
                        ysb[:, tt2, :512],
                        acc[:, :512],
                        mybir.ActivationFunctionType.Copy,
                    )
                    nc.vector.tensor_copy(ysb[:, tt2, 512:], acc[:, 512:])
                    eng.dma_start(out=y_d[tq, :, tt2, :], in_=ysb[:, tt2, :])

    nc.compile()
    return nc


def _route(xf, Wr, br):
    """Host router: exact top-2 + softmax weights (float64 for stable order)."""
    logits = xf.astype(np.float64) @ Wr.astype(np.float64) + br.astype(np.float64)
    order = np.argsort(-logits, axis=1, kind="stable")
    top2 = order[:, :TOP_K]  # [T, 2]
    v = np.take_along_axis(logits, top2, axis=1)
    v = v - v.max(axis=1, keepdims=True)
    ev = np.exp(v)
    rw = (ev / ev.sum(axis=1, keepdims=True)).astype(np.float32)  # [T, 2]
    return top2, rw


def _run(x, Wr, br, W1, b1, W2, b2, trace=False):
    B, S, d = x.shape
    T = B * S
    xf = np.ascontiguousarray(np.asarray(x, dtype=np.float32).reshape(T, d))

    top2, rw = _route(xf, Wr, br)

    token_lists = []
    weight_lists = []
    for e in range(E):
        in_slot0 = top2[:, 0] == e
        in_slot1 = top2[:, 1] == e
        toks = np.nonzero(in_slot0 | in_slot1)[0]
        w = np.where(in_slot0[toks], rw[toks, 0], rw[toks, 1]).astype(np.float32)
        token_lists.append(toks)
        weight_lists.append(w)

    # Capacity: balanced mean (rounded up to 256), capped by the SBUF
    # working set (x + h + W2 are resident). Pairs beyond it are computed
    # on the host - cheap for near-balanced routing.
    C = max(256, min(1024, -(-(T * TOP_K // E) // 256) * 256))
    spill_lists = [(t[C:], w[C:]) for t, w in zip(token_lists, weight_lists)]
    token_lists = [t[:C] for t in token_lists]
    weight_lists = [w[:C] for w in weight_lists]

    if C not in _compiled:
        _compiled[C] = _build(C)
    nc = _compiled[C]

    # Per-expert weight layouts (see _build DRAM shapes)
    W1 = np.asarray(W1, dtype=np.float32)
    W2 = np.asarray(W2, dtype=np.float32)
    b1 = np.asarray(b1, dtype=np.float32)
    b2 = np.asarray(b2, dtype=np.float32)
    w1h = np.ascontiguousarray(
        W1.reshape(E, DK, P, FT, P).transpose(0, 3, 2, 1, 4)
    ).astype(BF16)  # [E, FT, P(dp), DK, P(fi)]
    w2h = np.ascontiguousarray(W2.reshape(E, FT, P, D)).astype(BF16)  # [E, FT, P, D]
    b1h = np.ascontiguousarray(b1.reshape(E, FT, P).transpose(0, 2, 1))  # [E, P, FT]

    def pack(xg, c0, cn, dt=BF16):
        blk = xg[c0 : c0 + cn].T.reshape(DK, P, cn).transpose(1, 0, 2)
        return blk.reshape(P, DK * cn).astype(dt)

    cn0 = _token_chunks(C)[0][1]
    in_maps = []
    for e in range(E):
        toks = token_lists[e]
        xg = np.zeros((C, d), dtype=np.float32)
        xg[: len(toks)] = xf[toks]
        xt = np.empty((P, DK * C), dtype=BF16)
        for c0, cn in _token_chunks(C):
            xt[:, c0 * DK : c0 * DK + DK * cn] = pack(xg, c0, cn)
        # fp8e4m3 copy of block 0 for the startup-critical-path matmuls
        # (|x| <= ~5.2 sits in e4m3's native range; no scale needed)
        x8 = pack(xg, 0, cn0, dt=FP8)
        in_maps.append(
            {"xt": xt, "x8": x8, "w1": w1h[e], "w2": w2h[e], "b1": b1h[e]}
        )

    res = run_bass_kernel_spmd(
        nc, in_maps, core_ids=list(range(N_CORES)), trace=trace
    )

    # Host combine: out[t] = sum_k rw[t,k] * (y_{e_k}(t) + b2[e_k])
    w_dense = np.zeros((T, E), dtype=np.float32)
    np.put_along_axis(w_dense, top2, rw, axis=1)
    out = w_dense @ b2  # [T, D] bias part
    for e in range(E):
        toks = token_lists[e]
        yr = np.asarray(res.results[e]["y"], dtype=np.float32)  # [TT//2, P, 2, D]
        y = yr.transpose(0, 2, 1, 3).reshape(C, d)
        out[toks] += weight_lists[e][:, None] * y[: len(toks)]

    # Host-side spill: overflow pairs beyond the device capacity.
    try:
        from scipy.special import erf
    except ImportError:
        import math

        erf = np.vectorize(math.erf, otypes=[np.float32])

    sqrt2 = np.float32(np.sqrt(2.0))
    for e in range(E):
        toks, w = spill_lists[e]
        if len(toks) == 0:
            continue
        hs = xf[toks] @ W1[e] + b1[e]
        hs = 0.5 * hs * (1.0 + erf(hs / sqrt2))
        ys = hs @ W2[e]
        out[toks] += w[:, None] * ys

    return out.reshape(B, S, d).astype(np.float32), res


def kernel(x, Wr, br, W1, b1, W2, b2):
    out, _ = _run(x, Wr, br, W1, b1, W2, b2, trace=False)
    return out



# revision 46
# speedup vs baseline: 1.0194x; 1.0009x over previous
"""MoE (top-2 of 8 experts) Trainium2 kernel, expert-parallel across 8 NeuronCores.

Strategy (matches the expert-parallel sharding hint):
  - Host computes the router (logits -> top-2 -> softmax) and performs the
    token all-to-all: tokens are gathered per expert, padded to a common
    capacity C, and each core gets one expert's tokens + that expert's
    W1/b1/W2 weights.
  - Each core runs a Bass/Tile kernel computing
        y = gelu_exact(x @ W1 + b1) @ W2
    in bf16 (fp32 PSUM accumulate, ~3e-3 rel err, well under the 2e-2 gate).
  - Host scatter-adds the per-expert outputs back with the routing weights
    and adds sum_k w_k * b2[e_k] (folding b2 into the host combine).

Per-core dataflow (two phases, PE never idles between them):
  Phase A (h = gelu(x @ W1 + b1)): stationary = W1 128x128 blocks streamed
  from HBM, moving = x token blocks, all N=512 wide so the 150ns LDWEIGHTS
  stays hidden under 216ns matmuls; PSUM [f, 512 tok]; exact GELU +
  per-partition bias b1 fused into one ScalarE activation per tile; h kept
  RESIDENT in SBUF as bf16. Startup is bandwidth-walled (~240GB/s/core
  while all 8 cores pull their first bytes), so the critical transfers are
  demand-ordered across the sync/gpsimd/scalar DMA queues: x block 0 rides
  both rings as two dk-half TILES (the first 4 matmuls start when the lo
  half lands), w1_0 on the scalar ring, then a W1 ladder alternating
  rings; the first K=10 f-tiles run block 0 only, then their deferred
  block-1 groups (W1 held resident, W2 prefetch pulled into this no-new-
  data window), so x block 1's deadline sits past the bandwidth wall.
  Phase B (y = h @ W2): W2 fully resident in one SBUF tile (prefetched
  behind the W1 stream); stationary = h blocks [128 f, 128 tok], moving =
  W2 rows [128 f, 512 d]; each token pair's y accumulates over all 32
  f-tiles in dedicated PSUM banks, then drains (ScalarE+VectorE halves in
  parallel -> bf16 -> DMA) while the next pair accumulates; the last tile
  runs as three pieces (512/256/256 d-cols) in separate recycled PSUM
  tiles so only a 64KB drain+DMA sits in the program tail.
"""

import numpy as np
import ml_dtypes

import concourse.mybir as mybir
import concourse.tile as tile
from concourse import bacc
from concourse.bass_utils import run_bass_kernel_spmd

P = 128
D = 1024
F = 4096
E = 8
TOP_K = 2
DK = D // P   # 8 contraction tiles for GEMM1
FT = F // P   # 32 f tiles
N_CORES = 8

BF16 = ml_dtypes.bfloat16

_F32 = mybir.dt.float32
_BF16 = mybir.dt.bfloat16
_FP8 = mybir.dt.float8e4
FP8 = ml_dtypes.float8_e4m3

_compiled = {}  # C -> Bacc program


def _token_chunks(C):
    """Split C into 512-token chunks (PSUM-bank-width moving dim)."""
    chunks = []
    off = 0
    while off < C:
        cn = min(512, C - off)
        chunks.append((off, cn))
        off += cn
    return chunks


def _build(C):
    assert C % 256 == 0
    TT = C // P   # token tiles for GEMM2
    blocks = _token_chunks(C)   # 512-token blocks: h layout / phase B
    K = min(10, FT)             # f-tiles that run before x block 1 arrives
    nc = bacc.Bacc(None, target_bir_lowering=False)

    # x layout: dk-major 512-token blocks ([P, DK, 512] each, contiguous
    # per block so startup DMAs can slice dk-halves). x8 is an fp8e4m3 copy
    # of block 0 (half the bytes through the startup bandwidth wall): the
    # first K groups run stationary-bf16 x moving-fp8 matmuls (measured
    # exact vs cast on HW); ft>=K groups use the bf16 re-ship that arrives
    # off the critical path. fp8 x-quantization (~3.6% RMS/elem) on K/64
    # of h costs ~1e-2 final rel_l2 vs the 2e-2 gate.
    xt_d = nc.dram_tensor("xt", [P, DK * C], _BF16, kind="ExternalInput")
    cn0 = blocks[0][1]
    x8_d = nc.dram_tensor("x8", [P, DK * cn0], _FP8, kind="ExternalInput")
    w1_d = nc.dram_tensor("w1", [FT, P, DK, P], _BF16, kind="ExternalInput")
    w2_d = nc.dram_tensor("w2", [FT, P, D], _BF16, kind="ExternalInput")
    b1_d = nc.dram_tensor("b1", [P, FT], _F32, kind="ExternalInput")
    y_d = nc.dram_tensor("y", [TT // 2, P, 2, D], _BF16, kind="ExternalOutput")

    with tile.TileContext(nc) as tc:
        with (
            tc.tile_pool(name="xpool", bufs=1) as xpool,
            tc.tile_pool(name="cpool", bufs=1) as cpool,
            tc.tile_pool(name="w1pool", bufs=1) as w1pool,
            tc.tile_pool(name="w2pool", bufs=1) as w2pool,
            tc.tile_pool(name="hpool", bufs=1) as hpool,
            tc.tile_pool(name="ypool", bufs=2) as ypool,
            # 4 GEMM1 banks absorb ScalarE GELU / supply jitter three groups
            # deep; the sequential phase B reuses an acc tile only every
            # ~27us, so 2 ypsum tiles (4 banks) suffice. 4 + 4 = all 8 banks.
            tc.tile_pool(name="hpsum", bufs=4, space="PSUM") as hpsum,
            tc.tile_pool(name="ypsum", bufs=2, space="PSUM") as ypsum,
        ):
            # All inputs ride the sync DMA ring in demand order (the ring is
            # FIFO; emission order is preserved for these uniform triggers).
            # gate() pins a trigger behind the first x chunk via a WAW write
            # into its destination, so the list-scheduler cannot hoist it.
            def gate(dst_corner, src_tile):
                nc.vector.tensor_copy(dst_corner, src_tile[:, 0, 0:2])

            w1_live = K + 2  # first K held through their deferred groups
            solo = {}

            def w1_dma(ft, gated=False, eng=None, gate_src=None, split=False):
                t = w1pool.tile(
                    [P, DK, P], _BF16, tag="w1t", bufs=w1_live, name=f"w1s{ft}"
                )
                if gated or gate_src is not None:
                    gate(t[:, 0, 0:2], gate_src if gate_src is not None else x8t[0])
                if split:
                    # startup-critical: halves ride both DMA rings in parallel
                    h = DK // 2
                    nc.sync.dma_start(out=t[:, 0:h], in_=w1_d[ft][:, 0:h])
                    nc.gpsimd.dma_start(out=t[:, h:DK], in_=w1_d[ft][:, h:DK])
                else:
                    (eng or nc.sync).dma_start(out=t[:], in_=w1_d[ft])
                solo[ft] = t

            w2_sb = w2pool.tile([P, FT, D], _BF16, name="w2sb")
            w2_fill = [0]

            def w2_dma(gated=False):
                k = w2_fill[0]
                if k < FT:
                    if gated:
                        gate(w2_sb[:, k, 0:2], x8t[0])
                    nc.sync.dma_start(out=w2_sb[:, k], in_=w2_d[k])
                    w2_fill[0] = k + 1

            h_sb = [
                hpool.tile([P, FT, cn], _BF16, tag=f"hc{ci}", name=f"hc{ci}")
                for ci, (_, cn) in enumerate(blocks)
            ]

            # chunk list for GEMM1: (x-dram-offset, width, h-offset).
            # All chunks are full 512-token blocks: ring-split startup DMA
            # gets block 0 on-chip by ~11.5us, and N=512 keeps every group's
            # LDWEIGHTS hidden under 216ns matmuls (256-wide groups are
            # LDWEIGHTS-bound: 151ns load > 109ns matmul).
            xchunks = [(DK * c0, cn, c0) for c0, cn in blocks]
            xt_sb = {}
            x8t = []  # fp8 dk-half tiles of block 0

            def x8_dma():
                # Block 0 in fp8 as TWO dk-half tiles, each ring-split: the
                # first group's dk 0..3 matmuls start when the lo half (a
                # quarter of the bf16 block's bytes) lands; tile-granular
                # dependency tracking would otherwise hold them for all of
                # x0.
                hw = DK // 2 * cn0
                qw = hw // 2
                lo = xpool.tile([P, DK // 2, cn0], _FP8, tag="x8lo", name="x8lo")
                hi = xpool.tile([P, DK // 2, cn0], _FP8, tag="x8hi", name="x8hi")
                nc.sync.dma_start(out=lo[:, 0 : DK // 4], in_=x8_d[:, 0:qw])
                nc.gpsimd.dma_start(out=lo[:, DK // 4 :], in_=x8_d[:, qw:hw])
                nc.sync.dma_start(out=hi[:, 0 : DK // 4], in_=x8_d[:, hw : hw + qw])
                nc.gpsimd.dma_start(out=hi[:, DK // 4 :], in_=x8_d[:, hw + qw :])
                x8t.extend([lo, hi])

            def x_dma(si, gated=False, split=False):
                o, cn, _ = xchunks[si]
                t = xpool.tile([P, DK, cn], _BF16, tag=f"xt{si}", name=f"xt{si}")
                if gated:
                    gate(t[:, 0, 0:2], x8t[0])
                if split:
                    # halves ride both DMA rings in parallel
                    hw = DK // 2 * cn
                    nc.sync.dma_start(out=t[:, 0 : DK // 2], in_=xt_d[:, o : o + hw])
                    nc.gpsimd.dma_start(
                        out=t[:, DK // 2 : DK], in_=xt_d[:, o + hw : o + 2 * hw]
                    )
                else:
                    nc.sync.dma_start(out=t[:], in_=xt_d[:, o : o + DK * cn])
                xt_sb[si] = t

            def x_mov(si, dk, ft):
                """Moving operand for (chunk si, contraction tile dk)."""
                if si == 0 and ft < K:
                    t = x8t[0] if dk < DK // 2 else x8t[1]
                    return t[:, dk % (DK // 2), :]
                return xt_sb[si][:, dk, :]

            # Startup emission. Each DMA trigger costs ~0.6-0.7us of QUEUE
            # time, so the critical first wave is spread across FOUR queues
            # (scalar + vector are idle at startup and can trigger DMAs too):
            #   sync:   x0 lo-half      gpsimd: x0 hi-half
            #   scalar: w1_0 (then the dummy-GELU table load)
            #   vector: b1, w1_1 (then the warm memset + gates)
            # The first group's inputs (x0+w1_0+b1 = 1.28MB) then complete
            # at the aggregate-bandwidth floor (~11.5us); later tiles queue
            # FIFO behind them on the two rings.
            # Warm-tile memset FIRST on the gpsimd queue (runs ~6.2us, before
            # its DMA trigger) so the PE warm-up starts during engine init.
            warm = cpool.tile([P, 512], _BF16, tag="warm")
            nc.gpsimd.memset(warm[:], 0.0)
            w1_dma(0, eng=nc.scalar)
            b1_sb = cpool.tile([P, FT], _F32)
            nc.scalar.dma_start(out=b1_sb[:], in_=b1_d[:])
            x8_dma()
            # W1 ladder BEHIND x0's halves: w1_1 is split across both rings
            # (it is due only ~1.7us after x0 lands, sooner than a whole
            # tile can follow x0 on one ring); the rest alternate rings,
            # landing every ~1.1us (two rings at ~120GB/s each under the
            # 8-core startup contention) vs the 1.73us/tile consumption.
            # K=10 pushes x block 1's deadline past the ~240GB/s wall.
            w1_dma(1, split=True)
            for ft in range(2, K):
                w1_dma(ft, eng=nc.sync if ft % 2 == 0 else nc.gpsimd)
            # Dummy GELU on scratch: pulls ScalarE's ~1.3us ACT_TABLE_LOAD
            # into the startup DMA wait (scalar queue: w1_0 trigger, then
            # this). Otherwise the FIRST real GELU pays it, holds an hpsum
            # buffer longer, and stalls the PE through the rotation.
            scratch = cpool.tile([P, 16], _BF16, tag="scr")
            nc.scalar.activation(
                scratch[:],
                warm[:, 0:16],
                mybir.ActivationFunctionType.Gelu,
                bias=warm[:, 0:1],
                scale=1.0,
            )
            # PE warm-up: dummy zero matmuls with no DMA deps run during the
            # initial input-DMA wait, so the HAM clock gate reaches 2.4 GHz
            # before the real stream starts; sized to end at the measured
            # block-0 arrival (~11.5us).
            # Sized to cover the SLOWEST core's data arrival (~13us): the
            # max-core sets the graded time, and an idle gap before its
            # first real matmul also resets the HAM clock ramp (~2us of
            # 379ns mid-pstate matmuls). Fast cores just queue briefly.
            WARM = (6, 5)
            for r, nw in enumerate(WARM):
                pw = hpsum.tile([P, 512], _F32, tag="ph", name=f"pw{r}")
                for k in range(nw):
                    nc.tensor.matmul(
                        pw[:], warm[:, :P], warm[:], start=(k == 0), stop=(k == nw - 1)
                    )
            # x block 1 follows the W1 ladder on both rings (ring FIFO
            # orders the transfers; the gate pins emission order against
            # list-scheduler hoisting).
            if len(xchunks) > 1:
                x_dma(1, split=True, gated=True)
            # bf16 re-ship of block 0 for the ft>=K groups (due ~17us after
            # the fp8 copy; rides behind x block 1 on both rings)
            x_dma(0, split=True, gated=True)

            def gemm1_group(ft, si):
                _, cn, h0 = xchunks[si]
                ph = hpsum.tile([P, 512], _F32, tag="ph")
                for dk in range(DK):
                    nc.tensor.matmul(
                        ph[:, :cn],
                        solo[ft][:, dk, :],
                        x_mov(si, dk, ft),
                        start=(dk == 0),
                        stop=(dk == DK - 1),
                    )
                nc.scalar.activation(
                    h_sb[h0 // 512][:, ft, h0 % 512 : h0 % 512 + cn],
                    ph[:, :cn],
                    mybir.ActivationFunctionType.Gelu,
                    bias=b1_sb[:, ft : ft + 1],
                    scale=1.0,
                )

            # Phase A order: the first K f-tiles run block 0 while block 1
            # streams in, then their deferred block-1 groups (W1 tiles held
            # resident); the rest run f-tile-major over both blocks so each
            # W1 tile is streamed exactly once.
            NB = len(xchunks)
            order = [(ft, 0) for ft in range(K)]
            for b in range(1, NB):
                order += [(ft, b) for ft in range(K)]
            order += [(ft, b) for ft in range(K, FT) for b in range(NB)]

            seen = set(ft for ft, _ in order[:K])
            for ft, si in order:
                if ft not in solo:
                    w1_dma(ft, gated=True)
                if ft not in seen:
                    seen.add(ft)
                    # W1 lookahead + W2 prefetch ride the same ring.
                    la = ft + 2
                    if la < FT and la not in solo:
                        w1_dma(la, gated=True)
                    w2_dma(gated=w2_fill[0] < 2)
                    w2_dma(gated=w2_fill[0] < 2)
                elif si >= 1 and ft < K:
                    # deferred-block groups consume no new data: use their
                    # window to pull the W2 prefetch forward
                    w2_dma(gated=w2_fill[0] < 2)
                gemm1_group(ft, si)
            while w2_fill[0] < FT:
                w2_dma()

            # Phase B: token tiles, full 32-step PSUM accumulation each.
            # The two tiles of a pair run SEQUENTIALLY (not ft-interleaved),
            # so each tile's drain + y DMA overlaps the next tile's 64-MM
            # accumulation; only the very last tile's drain+DMA lands in the
            # program tail.
            for tq in range(TT // 2):
                ci = (tq * 2 * P) // 512  # block holding this token pair
                cb = tq * 2 * P - blocks[ci][0]  # base token within block
                ysb = ypool.tile([P, 2, D], _BF16, tag="ysb")
                for tt2 in range(2):
                    acc = ypsum.tile([P, D], _F32, tag="py", name=f"py{tq}_{tt2}")
                    eng = nc.sync if (tq * 2 + tt2) % 2 == 0 else nc.gpsimd
                    if tq == TT // 2 - 1 and tt2 == 1:
                        # Final token tile: d-half 0, then two d-quarters as
                        # separate sequential accumulation groups, so the
                        # program-tail drain + y DMA is only 64KB (the drains
                        # of the earlier pieces hide under later matmuls).
                        # Each piece gets its OWN PSUM tile (recycled from the
                        # idle phase-A pool): slicing one shared acc tile puts
                        # a false WAR between piece N's drain-read and piece
                        # N+1's first matmul (~0.8us PE stall each, measured).
                        pieces = [(0, 512), (512, 256), (768, 256)]
                        for pi, (d0, dn) in enumerate(pieces):
                            pt = hpsum.tile([P, 512], _F32, tag="ph", name=f"pyf{pi}")
                            for ft in range(FT):
                                hblk = h_sb[ci][
                                    :, ft, cb + tt2 * P : cb + (tt2 + 1) * P
                                ]
                                nc.tensor.matmul(
                                    pt[:, :dn],
                                    hblk,
                                    w2_sb[:, ft, d0 : d0 + dn],
                                    start=(ft == 0),
                                    stop=(ft == FT - 1),
                                )
                            if pi == len(pieces) - 1:
                                # Last piece: drain + ship as two parallel
                                # halves (Scalar+sync / Vector+gpsimd) to
                                # halve the program-tail serial chain.
                                hn = dn // 2
                                nc.scalar.activation(
                                    ysb[:, tt2, d0 : d0 + hn],
                                    pt[:, :hn],
                                    mybir.ActivationFunctionType.Copy,
                                )
                                nc.sync.dma_start(
                                    out=y_d[tq, :, tt2, d0 : d0 + hn],
                                    in_=ysb[:, tt2, d0 : d0 + hn],
                                )
                                nc.vector.tensor_copy(
                                    ysb[:, tt2, d0 + hn : d0 + dn],
                                    pt[:, hn:dn],
                                )
                                nc.gpsimd.dma_start(
                                    out=y_d[tq, :, tt2, d0 + hn : d0 + dn],
                                    in_=ysb[:, tt2, d0 + hn : d0 + dn],
                                )
                            elif pi % 2 == 0:
                                nc.scalar.activation(
                                    ysb[:, tt2, d0 : d0 + dn],
                                    pt[:, :dn],
                                    mybir.ActivationFunctionType.Copy,
                                )
                                nc.sync.dma_start(
                                    out=y_d[tq, :, tt2, d0 : d0 + dn],
                                    in_=ysb[:, tt2, d0 : d0 + dn],
                                )
                            else:
                                nc.vector.tensor_copy(
                                    ysb[:, tt2, d0 : d0 + dn],
                                    pt[:, :dn],
                                )
                                nc.gpsimd.dma_start(
                                    out=y_d[tq, :, tt2, d0 : d0 + dn],
                                    in_=ysb[:, tt2, d0 : d0 + dn],
                                )
                        continue
                    for ft in range(FT):
                        hblk = h_sb[ci][:, ft, cb + tt2 * P : cb + (tt2 + 1) * P]
                        for dh in range(2):
                            nc.tensor.matmul(
                                acc[:, dh * 512 : (dh + 1) * 512],
                                hblk,
                                w2_sb[:, ft, dh * 512 : (dh + 1) * 512],
                                start=(ft == 0),
                                stop=(ft == FT - 1),
                            )
                    # Drain the two PSUM banks in parallel on Scalar+Vector,
                    # then ship this token tile immediately.
                    nc.scalar.activation(
                        ysb[:, tt2, :512],
                        acc[:, :512],
                        mybir.ActivationFunctionType.Copy,
                    )
                    nc.vector.tensor_copy(ysb[:, tt2, 512:], acc[:, 512:])
                    eng.dma_start(out=y_d[tq, :, tt2, :], in_=ysb[:, tt2, :])

    nc.compile()
    return nc


def _route(xf, Wr, br):
    """Host router: exact top-2 + softmax weights (float64 for stable order)."""
    logits = xf.astype(np.float64) @ Wr.astype(np.float64) + br.astype(np.float64)
    order = np.argsort(-logits, axis=1, kind="stable")
    top2 = order[:, :TOP_K]  # [T, 2]
    v = np.take_along_axis(logits, top2, axis=1)
    v = v - v.max(axis=1, keepdims=True)
    ev = np.exp(v)
    rw = (ev / ev.sum(axis=1, keepdims=True)).astype(np.float32)  # [T, 2]
    return top2, rw


def _run(x, Wr, br, W1, b1, W2, b2, trace=False):
    B, S, d = x.shape
    T = B * S
    xf = np.ascontiguousarray(np.asarray(x, dtype=np.float32).reshape(T, d))

    top2, rw = _route(xf, Wr, br)

    token_lists = []
    weight_lists = []
    for e in range(E):
        in_slot0 = top2[:, 0] == e
        in_slot1 = top2[:, 1] == e
        toks = np.nonzero(in_slot0 | in_slot1)[0]
        w = np.where(in_slot0[toks], rw[toks, 0], rw[toks, 1]).astype(np.float32)
        token_lists.append(toks)
        weight_lists.append(w)

    # Capacity: balanced mean (rounded up to 256), capped by the SBUF
    # working set (x + h + W2 are resident). Pairs beyond it are computed
    # on the host - cheap for near-balanced routing.
    C = max(256, min(1024, -(-(T * TOP_K // E) // 256) * 256))
    spill_lists = [(t[C:], w[C:]) for t, w in zip(token_lists, weight_lists)]
    token_lists = [t[:C] for t in token_lists]
    weight_lists = [w[:C] for w in weight_lists]

    if C not in _compiled:
        _compiled[C] = _build(C)
    nc = _compiled[C]

    # Per-expert weight layouts (see _build DRAM shapes)
    W1 = np.asarray(W1, dtype=np.float32)
    W2 = np.asarray(W2, dtype=np.float32)
    b1 = np.asarray(b1, dtype=np.float32)
    b2 = np.asarray(b2, dtype=np.float32)
    w1h = np.ascontiguousarray(
        W1.reshape(E, DK, P, FT, P).transpose(0, 3, 2, 1, 4)
    ).astype(BF16)  # [E, FT, P(dp), DK, P(fi)]
    w2h = np.ascontiguousarray(W2.reshape(E, FT, P, D)).astype(BF16)  # [E, FT, P, D]
    b1h = np.ascontiguousarray(b1.reshape(E, FT, P).transpose(0, 2, 1))  # [E, P, FT]

    def pack(xg, c0, cn, dt=BF16):
        blk = xg[c0 : c0 + cn].T.reshape(DK, P, cn).transpose(1, 0, 2)
        return blk.reshape(P, DK * cn).astype(dt)

    cn0 = _token_chunks(C)[0][1]
    in_maps = []
    for e in range(E):
        toks = token_lists[e]
        xg = np.zeros((C, d), dtype=np.float32)
        xg[: len(toks)] = xf[toks]
        xt = np.empty((P, DK * C), dtype=BF16)
        for c0, cn in _token_chunks(C):
            xt[:, c0 * DK : c0 * DK + DK * cn] = pack(xg, c0, cn)
        # fp8e4m3 copy of block 0 for the startup-critical-path matmuls
        # (|x| <= ~5.2 sits in e4m3's native range; no scale needed)
        x8 = pack(xg, 0, cn0, dt=FP8)
        in_maps.append(
            {"xt": xt, "x8": x8, "w1": w1h[e], "w2": w2h[e], "b1": b1h[e]}
        )

    res = run_bass_kernel_spmd(
        nc, in_maps, core_ids=list(range(N_CORES)), trace=trace
    )

    # Host combine: out[t] = sum_k rw[t,k] * (y_{e_k}(t) + b2[e_k])
    w_dense = np.zeros((T, E), dtype=np.float32)
    np.put_along_axis(w_dense, top2, rw, axis=1)
    out = w_dense @ b2  # [T, D] bias part
    for e in range(E):
        toks = token_lists[e]
        yr = np.asarray(res.results[e]["y"], dtype=np.float32)  # [TT//2, P, 2, D]
        y = yr.transpose(0, 2, 1, 3).reshape(C, d)
        out[toks] += weight_lists[e][:, None] * y[: len(toks)]

    # Host-side spill: overflow pairs beyond the device capacity.
    try:
        from scipy.special import erf
    except ImportError:
        import math

        erf = np.vectorize(math.erf, otypes=[np.float32])

    sqrt2 = np.float32(np.sqrt(2.0))
    for e in range(E):
        toks, w = spill_lists[e]
        if len(toks) == 0:
            continue
        hs = xf[toks] @ W1[e] + b1[e]
        hs = 0.5 * hs * (1.0 + erf(hs / sqrt2))
        ys = hs @ W2[e]
        out[toks] += w[:, None] * ys

    return out.reshape(B, S, d).astype(np.float32), res


def kernel(x, Wr, br, W1, b1, W2, b2):
    out, _ = _run(x, Wr, br, W1, b1, W2, b2, trace=False)
    return out

